# revision 23
# baseline (speedup 1.0000x reference)
"""GuidedAttentionLoss on Trainium2 — 8 NeuronCores, diagonal-band gather.

loss = mean(attention_weights * mask), mask[b,i,j] =
    (i < out_len_b) & (j < in_len_b) ? exp(-(j - floor(i/out*in))^2 / (2*0.4^2)) : 0

With sigma=0.4 the Gaussian underflows to exactly 0 in f32 beyond
|j - ideal_i| ~ 4.6, so per valid row only a ~9-wide band of columns can
contribute. Strategy:

- Batches are partitioned into 8 slot-columns of 8 (one batch per core per
  slot) -> pure SPMD: every core runs the identical program; per-core data
  (attention shard + center tables) differs. A hill-climb swaps members
  between columns to cluster similar (slope, out_len).
- Each column is split (exact DP) into row segments; per segment a quantized
  shear line sigma(i) = a2*(i%128) + at*(i//128) + b tracks ideal(i), and a
  single 3-dim DMA access pattern [[400+a2,128],[51200+at,nt],[1,W]] gathers
  the whole segment's band in ONE DMA instruction. W is fitted exactly from
  the union of the live members' needs (invalid rows get center=+1e4).
- Mask math per segment over the [128, nt*W] tile:
    path A: d2[:, t] = ACT Square(-w + center_t)   (per-tile bias, no sub)
    path B: d = w - center (DVE, broadcast APs); d2 = ACT Square(d)
    g = ACT Exp(-3.125*d2);  acc[:,s] += g*attn  (DVE stt accum)
  Paths are assigned per segment to balance ACT vs DVE.
- Out-of-range garbage (front spill j<0, j >= min(in,400)) is not masked on
  device; the host subtracts those few boundary terms exactly in f64.
"""

import numpy as np

import concourse.bacc as bacc
import concourse.bass as bass  # noqa: F401
import concourse.mybir as mybir
from concourse.ap import AP
from concourse import tile
from concourse.bass_utils import run_bass_kernel_spmd

N_CORES = 8
B, T, E = 64, 2000, 400
B_LOC = B // N_CORES
P = 128
D = 4
PADF = 512
PADB = 81920
FLAT = PADF + B_LOC * T * E + PADB
NEG_SCALE = -3.125
F32 = mybir.dt.float32
AF = mybir.ActivationFunctionType
OP = mybir.AluOpType

SEG_FIXED_NS = 1300.0
SWAP_ITERS = 400

_NC_CACHE = {}


def _ideal_f32(i, in_len, out_len):
    safe_out = np.float32(max(float(out_len), 1.0))
    return np.floor((i.astype(np.float32) / safe_out) * np.float32(in_len)).astype(
        np.float32
    )


def _dma_row_ns(W):
    by = 4 * W
    mult = 2.0 if by < 512 else 1.0
    return max(by * mult / 22.5, 7.0)


class _Seg:
    __slots__ = ("g", "members", "t0", "nt", "W", "a2", "at", "b",
                 "sigma", "path_a")

    def key(self):
        return (self.g, self.t0, self.nt, self.W, self.a2, self.at, self.b,
                self.path_a)


class _ColCtx:
    """Precomputed per-column row data for interval fitting."""

    def __init__(self, members, il, ol):
        self.members = members
        self.max_out = max(int(ol[b]) for b in members)
        self.ntt = (min(self.max_out, T) + P - 1) // P
        rows = self.ntt * P
        i = np.arange(rows)
        A = np.full((8, rows), 1e9)
        Bb = np.full((8, rows), -1e9)
        valid = np.zeros((8, rows), bool)
        for m, b in enumerate(members):
            o, n = int(ol[b]), int(il[b])
            valid[m] = i < min(o, T)
            idl = _ideal_f32(i, n, o).astype(np.float64)
            A[m] = np.maximum(0.0, idl - D)
            Bb[m] = np.minimum(n - 1, idl + D)
        self.anyv = valid.any(0)
        self.Amin = np.where(valid, A, 1e9).min(0)
        self.Bmax = np.where(valid, Bb, -1e9).max(0)
        self.slopes = np.array([il[b] / max(ol[b], 1) for b in members])
        self.valid = valid

    def fit(self, t0, t1):
        """Best (W, a2, at, b) for tiles [t0, t1). Returns None if no valid
        rows (segment can be dropped)."""
        rows = (t1 - t0) * P
        sl = slice(t0 * P, t1 * P)
        anyv = self.anyv[sl]
        if not anyv.any():
            return None
        Amin = self.Amin[sl]
        Bmax = self.Bmax[sl]
        live = self.valid[:, sl].any(1)
        ss = self.slopes[live]
        rr = np.arange(rows)
        t_idx = rr // P
        p = rr % P
        cands = set()
        for s in set(np.quantile(ss, [0.0, 0.25, 0.5, 0.75, 1.0])):
            for f1 in (np.floor, np.round):
                a2 = int(f1(s))
                for f3 in (np.floor, np.round):
                    at3 = int(f3(128 * s))
                    for dat in (-1, 0, 1):
                        cands.add((a2, at3 + dat))
        cands.add((0, 0))
        best = None
        for a2, at in cands:
            sig0 = a2 * p + at * t_idx
            b_off = int(np.floor((Amin - sig0)[anyv].min()))
            W = int(np.ceil((Bmax - sig0)[anyv].max() - b_off)) + 1
            if best is None or W < best[0]:
                best = (W, a2, at, b_off)
        return best

    def seg_cost(self, t0, t1):
        f = self.fit(t0, t1)
        if f is None:
            return 0.0, None
        W = f[0]
        nt = t1 - t0
        fw = nt * W
        dma = nt * P * _dma_row_ns(W) / 16.0
        cost = max(dma, 1.39 * fw) + 0.9 * fw + SEG_FIXED_NS
        return cost, f

    def plan(self, max_segs=6):
        """DP split of [0, ntt) into segments minimizing total cost."""
        nt = self.ntt
        # interval costs
        icost = {}
        ifit = {}
        for t0 in range(nt):
            for t1 in range(t0 + 1, nt + 1):
                c, f = self.seg_cost(t0, t1)
                icost[(t0, t1)] = c
                ifit[(t0, t1)] = f
        INF = float("inf")
        dp = [INF] * (nt + 1)
        prev = [0] * (nt + 1)
        dp[0] = 0.0
        for t1 in range(1, nt + 1):
            for t0 in range(t1):
                c = dp[t0] + icost[(t0, t1)]
                if c < dp[t1]:
                    dp[t1] = c
                    prev[t1] = t0
        cuts = []
        t = nt
        while t > 0:
            t0 = prev[t]
            cuts.append((t0, t))
            t = t0
        cuts.reverse()
        out = []
        for t0, t1 in cuts:
            f = ifit[(t0, t1)]
            if f is None:
                continue
            out.append((t0, t1, f))
        return dp[nt], out

    def quick_cost(self):
        return self.seg_cost(0, self.ntt)[0]


def _build_schedule(input_lengths, output_lengths):
    il = np.asarray(input_lengths, dtype=np.int64)
    ol = np.asarray(output_lengths, dtype=np.int64)
    slopes = il.astype(np.float64) / np.maximum(ol, 1)
    order = np.argsort(slopes, kind="stable")
    groups = [[int(order[8 * g + c]) for c in range(8)] for g in range(8)]

    # hill-climb member swaps on the quick (single-segment) cost
    ctxs = [_ColCtx(gr, il, ol) for gr in groups]
    costs = [c.quick_cost() for c in ctxs]
    rng = np.random.RandomState(0)
    for _ in range(SWAP_ITERS):
        g1, g2 = rng.randint(0, 8, 2)
        if g1 == g2:
            continue
        m1, m2 = rng.randint(0, 8, 2)
        a, bb = groups[g1][m1], groups[g2][m2]
        n1 = groups[g1].copy()
        n2 = groups[g2].copy()
        n1[m1], n2[m2] = bb, a
        c1 = _ColCtx(n1, il, ol)
        c2 = _ColCtx(n2, il, ol)
        if c1.quick_cost() + c2.quick_cost() < costs[g1] + costs[g2] - 1.0:
            groups[g1], groups[g2] = n1, n2
            ctxs[g1], ctxs[g2] = c1, c2
            costs[g1], costs[g2] = c1.quick_cost(), c2.quick_cost()

    assign = [[groups[g][c] for g in range(8)] for c in range(8)]
    segs = []
    for g in range(8):
        ctx = ctxs[g]
        _, plan = ctx.plan()
        for t0, t1, (W, a2, at, b_off) in plan:
            seg = _Seg()
            seg.g = g
            seg.members = groups[g]
            seg.t0 = t0
            seg.nt = t1 - t0
            seg.W = W
            seg.a2 = a2
            seg.at = at
            seg.b = b_off
            rr = np.arange(seg.nt * P)
            seg.sigma = a2 * (rr % P) + at * (rr // P) + b_off
            segs.append(seg)

    # path A/B assignment balancing ACT vs DVE
    act_ns = dve_ns = 0.0
    for seg in sorted(segs, key=lambda s: -s.nt * s.W):
        fw = seg.nt * seg.W
        a_act = 1.67 * fw + 192 * seg.nt + 0.833 * fw
        a_dve = 1.042 * fw
        b_act = 0.833 * fw * 2
        b_dve = 2.084 * fw
        if max(act_ns + a_act, dve_ns + a_dve) <= max(
            act_ns + b_act, dve_ns + b_dve
        ):
            seg.path_a = True
            act_ns += a_act
            dve_ns += a_dve
        else:
            seg.path_a = False
            act_ns += b_act
            dve_ns += b_dve

    _coverage_check(segs, il, ol)
    return assign, segs


def _coverage_check(segs, il, ol):
    covered = {}
    for seg in segs:
        rows = seg.nt * P
        i = seg.t0 * P + np.arange(rows)
        for m, b in enumerate(seg.members):
            o, n = int(ol[b]), int(il[b])
            v = i < min(o, T)
            if not v.any():
                continue
            idl = _ideal_f32(i, n, o).astype(np.float64)
            A = np.maximum(0.0, idl - D)
            Bb = np.minimum(n - 1, idl + D)
            ok = (~v) | ((seg.sigma <= A) & (Bb < seg.sigma + seg.W))
            assert ok.all(), (seg.g, b, np.where(~ok)[0][:5])
            base = seg.g * T * E + i * E + seg.sigma
            assert (PADF + base).min() >= 0
            assert (PADF + base + seg.W).max() <= FLAT
            cv = covered.setdefault((seg.g, b), np.zeros(T, bool))
            cv[np.clip(i[v], 0, T - 1)] = True
    # every valid row of every batch must be covered by some segment
    seen = {}
    for seg in segs:
        for b in seg.members:
            seen[(seg.g, b)] = True
    for (g, b) in seen:
        o = min(int(ol[b]), T)
        cv = covered.get((g, b), np.zeros(T, bool))
        assert cv[:o].all(), (g, b, int(np.argmin(cv[:o])))


def _build_nc(segs):
    ntt = sum(s.nt for s in segs)
    nseg = len(segs)
    nc = bacc.Bacc(None, target_bir_lowering=False)
    attn = nc.declare_dram_parameter("attn", [FLAT], F32, isOutput=False)
    center_d = nc.declare_dram_parameter("center", [P, ntt], F32, isOutput=False)
    acc_d = nc.declare_dram_parameter("acc", [P, nseg], F32, isOutput=True)

    with tile.TileContext(nc) as tc:
        with (
            tc.tile_pool(name="const", bufs=1) as const_pool,
            tc.tile_pool(name="at", bufs=3) as at_pool,
            tc.tile_pool(name="d", bufs=3) as d_pool,
            tc.tile_pool(name="g", bufs=3) as g_pool,
            tc.tile_pool(name="junk", bufs=3) as junk_pool,
        ):
            w_i32 = const_pool.tile([P, E], mybir.dt.int32, tag="w_i32")
            w_f32 = const_pool.tile([P, E], F32, tag="w_f32")
            center = const_pool.tile([P, ntt], F32, tag="center")
            acc = const_pool.tile([P, nseg], F32, tag="acc")

            nc.gpsimd.iota(w_i32[:], pattern=[[1, E]], base=0, channel_multiplier=0)
            nc.vector.tensor_copy(w_f32[:], w_i32[:])
            nc.gpsimd.memset(acc[:], 0.0)

            # attn DMAs are issued first so transfers start while tables load
            ats = []
            for seg in segs:
                nt, W = seg.nt, seg.W
                at = at_pool.tile([P, nt * W], F32, tag="at")
                src = AP(
                    attn[:].tensor,
                    PADF + seg.g * T * E + seg.t0 * P * E + seg.b,
                    [[E + seg.a2, P], [P * E + seg.at, nt], [1, W]],
                )
                dst = at[:]
                nc.sync.dma_start(
                    out=AP(dst.tensor, dst.offset, [dst.ap[0], [W, nt], [1, W]]),
                    in_=src,
                )
                ats.append(at)
            nc.sync.dma_start(out=center[:], in_=center_d[:])

            k0 = 0
            for si, seg in enumerate(segs):
                nt, W = seg.nt, seg.W
                fw = nt * W
                at = ats[si]
                d2 = junk_pool.tile([P, fw], F32, tag="d2")
                if seg.path_a:
                    for t in range(nt):
                        nc.scalar.activation(
                            d2[:, t * W : (t + 1) * W],
                            w_f32[:, 0:W],
                            AF.Square,
                            bias=center[:, k0 + t : k0 + t + 1],
                            scale=-1.0,
                        )
                else:
                    d = d_pool.tile([P, fw], F32, tag="d")
                    wap = w_f32[:, 0:W]
                    w_b = AP(wap.tensor, wap.offset, [wap.ap[0], [0, nt], [1, W]])
                    cap = center[:, k0 : k0 + nt]
                    c_b = AP(cap.tensor, cap.offset, [cap.ap[0], [1, nt], [0, W]])
                    dap = d[:]
                    d3 = AP(dap.tensor, dap.offset, [dap.ap[0], [W, nt], [1, W]])
                    nc.vector.tensor_tensor(d3, w_b, c_b, OP.subtract)
                    nc.scalar.activation(d2[:], d[:], AF.Square)
                gt = g_pool.tile([P, fw], F32, tag="gt")
                nc.scalar.activation(gt[:], d2[:], AF.Exp, scale=NEG_SCALE)
                jk = junk_pool.tile([P, fw], F32, tag="jk")
                nc.vector.scalar_tensor_tensor(
                    jk[:], gt[:], 1.0, at[:], OP.mult, OP.mult,
                    accum_out=acc[:, si : si + 1],
                )
                k0 += nt
            nc.sync.dma_start(out=acc_d[:], in_=acc[:])
    return nc


def _get_nc(segs):
    key = tuple(s.key() for s in segs)
    if key not in _NC_CACHE:
        nc = _build_nc(segs)
        if not nc.is_finalized():
            nc.finalize()
        _NC_CACHE[key] = nc
    return _NC_CACHE[key]


def _make_tables(il, ol, assign_c, segs):
    ntt = sum(s.nt for s in segs)
    center = np.full((P, ntt), 1e4, np.float32)
    k0 = 0
    for seg in segs:
        b = assign_c[seg.g]
        o, n = int(ol[b]), int(il[b])
        rows = seg.nt * P
        i = seg.t0 * P + np.arange(rows)
        idl = _ideal_f32(i, n, o)
        validr = i < min(o, T)
        cen = np.where(validr, idl - seg.sigma.astype(np.float32), np.float32(1e4))
        center[:, k0 : k0 + seg.nt] = cen.reshape(seg.nt, P).T
        k0 += seg.nt
    return {"center": center}


def _garbage_correction(in_maps, il, ol, assign, segs):
    """Sum of out-of-range contributions the device wrongly included."""
    M = 24
    corr = 0.0
    for c in range(N_CORES):
        flat = in_maps[c]["attn"]
        for seg in segs:
            b = assign[c][seg.g]
            o, n = int(ol[b]), int(il[b])
            lim = min(n, E)
            rows = seg.nt * P
            i = seg.t0 * P + np.arange(rows)
            validr = i < min(o, T)
            idl = _ideal_f32(i, n, o).astype(np.float64)
            sg = seg.sigma
            fr = validr & (
                ((sg < 0) & (idl <= M)) | ((sg + seg.W > lim) & (idl >= lim - M))
            )
            if not fr.any():
                continue
            ii = i[fr]
            j = sg[fr][:, None] + np.arange(seg.W)[None, :]
            dd = j - idl[fr][:, None]
            bad = ((j < 0) | (j >= lim)) & (np.abs(dd) <= M)
            if not bad.any():
                continue
            addr = PADF + seg.g * T * E + ii[:, None] * E + j
            vals = flat[addr[bad]].astype(np.float64)
            corr += float(np.sum(np.exp(-3.125 * dd[bad] ** 2) * vals))
    return corr


def _run(attention_weights, input_lengths, output_lengths, **spmd_kwargs):
    attention_weights = np.ascontiguousarray(attention_weights, dtype=np.float32)
    il = np.asarray(input_lengths, dtype=np.int64)
    ol = np.asarray(output_lengths, dtype=np.int64)
    assign, segs = _build_schedule(il, ol)
    in_maps = []
    for c in range(N_CORES):
        flat = np.empty(FLAT, np.float32)
        flat[:PADF] = 0.0
        flat[PADF : PADF + B_LOC * T * E] = attention_weights[assign[c]].reshape(-1)
        flat[PADF + B_LOC * T * E :] = 0.0
        in_maps.append({"attn": flat, **_make_tables(il, ol, assign[c], segs)})
    res = run_bass_kernel_spmd(
        _get_nc(segs), in_maps, list(range(N_CORES)), **spmd_kwargs
    )
    total = sum(float(r["acc"].sum(dtype=np.float64)) for r in res.results)
    total -= _garbage_correction(in_maps, il, ol, assign, segs)
    return np.float32(total / float(B * T * E)), res


def kernel(attention_weights, input_lengths, output_lengths):
    out, _ = _run(attention_weights, input_lengths, output_lengths)
    return out


# revision 28
# speedup vs baseline: 1.2195x; 1.2195x over previous
"""GuidedAttentionLoss on Trainium2 — 8 NeuronCores, diagonal-band gather.

loss = mean(attention_weights * mask), mask[b,i,j] =
    (i < out_len_b) & (j < in_len_b) ? exp(-(j - floor(i/out*in))^2 / (2*0.4^2)) : 0

With sigma=0.4 the Gaussian underflows to exactly 0 in f32 beyond
|j - ideal_i| ~ 4.6, so per valid row only a ~9-wide band of columns can
contribute. Strategy:

- Batches are partitioned into 8 slot-columns of 8 (one batch per core per
  slot) -> pure SPMD: every core runs the identical program; per-core data
  (attention shard + center tables) differs. A hill-climb swaps members
  between columns to cluster similar (slope, out_len).
- Each column is split (exact DP) into row segments; per segment a quantized
  shear line sigma(i) = a2*(i%128) + at*(i//128) + b tracks ideal(i), and a
  single 3-dim DMA access pattern [[400+a2,128],[51200+at,nt],[1,W]] gathers
  the whole segment's band in ONE DMA instruction. W is fitted exactly from
  the union of the live members' needs (invalid rows get center=+1e4).
- Mask math per segment over the [128, nt*W] tile:
    path A: d2[:, t] = ACT Square(-w + center_t)   (per-tile bias, no sub)
    path B: d = w - center (DVE, broadcast APs); d2 = ACT Square(d)
    g = ACT Exp(-3.125*d2);  acc[:,s] += g*attn  (DVE stt accum)
  Paths are assigned per segment to balance ACT vs DVE.
- Out-of-range garbage (front spill j<0, j >= min(in,400)) is not masked on
  device; the host subtracts those few boundary terms exactly in f64.
"""

import numpy as np

import concourse.bacc as bacc
import concourse.bass as bass  # noqa: F401
import concourse.mybir as mybir
from concourse.ap import AP
from concourse import tile
from concourse.bass_utils import run_bass_kernel_spmd

N_CORES = 8
B, T, E = 64, 2000, 400
B_LOC = B // N_CORES
P = 128
D = 4
PADF = 512
PADB = 81920
FLAT = PADF + B_LOC * T * E + PADB
NEG_SCALE = -3.125
F32 = mybir.dt.float32
AF = mybir.ActivationFunctionType
OP = mybir.AluOpType

SEG_FIXED_NS = 900.0
SWAP_ITERS = 1500

_NC_CACHE = {}


def _ideal_f32(i, in_len, out_len):
    safe_out = np.float32(max(float(out_len), 1.0))
    return np.floor((i.astype(np.float32) / safe_out) * np.float32(in_len)).astype(
        np.float32
    )


def _dma_row_ns(W):
    by = 4 * W
    mult = 2.0 if by < 512 else 1.0
    return max(by * mult / 22.5, 7.0)


class _Seg:
    __slots__ = ("g", "members", "t0", "nt", "W", "a2", "at", "b",
                 "sigma", "path_a")

    def key(self):
        return (self.g, self.t0, self.nt, self.W, self.a2, self.at, self.b,
                self.path_a)


class _ColCtx:
    """Precomputed per-column row data for interval fitting."""

    def __init__(self, members, il, ol):
        self.members = members
        self.max_out = max(int(ol[b]) for b in members)
        self.ntt = (min(self.max_out, T) + P - 1) // P
        rows = self.ntt * P
        i = np.arange(rows)
        A = np.full((8, rows), 1e9)
        Bb = np.full((8, rows), -1e9)
        valid = np.zeros((8, rows), bool)
        for m, b in enumerate(members):
            o, n = int(ol[b]), int(il[b])
            valid[m] = i < min(o, T)
            idl = _ideal_f32(i, n, o).astype(np.float64)
            A[m] = np.maximum(0.0, idl - D)
            Bb[m] = np.minimum(n - 1, idl + D)
        self.anyv = valid.any(0)
        self.Amin = np.where(valid, A, 1e9).min(0)
        self.Bmax = np.where(valid, Bb, -1e9).max(0)
        self.slopes = np.array([il[b] / max(ol[b], 1) for b in members])
        self.valid = valid

    def fit(self, t0, t1):
        """Best (W, a2, at, b) for tiles [t0, t1). Returns None if no valid
        rows (segment can be dropped)."""
        rows = (t1 - t0) * P
        sl = slice(t0 * P, t1 * P)
        anyv = self.anyv[sl]
        if not anyv.any():
            return None
        Amin = self.Amin[sl]
        Bmax = self.Bmax[sl]
        live = self.valid[:, sl].any(1)
        ss = self.slopes[live]
        rr = np.arange(rows)
        t_idx = rr // P
        p = rr % P
        cands = set()
        for s in set(np.quantile(ss, [0.0, 0.25, 0.5, 0.75, 1.0])):
            for f1 in (np.floor, np.round):
                a2 = int(f1(s))
                for f3 in (np.floor, np.round):
                    at3 = int(f3(128 * s))
                    for dat in (-1, 0, 1):
                        cands.add((a2, at3 + dat))
        cands.add((0, 0))
        best = None
        for a2, at in cands:
            sig0 = a2 * p + at * t_idx
            b_off = int(np.floor((Amin - sig0)[anyv].min()))
            W = int(np.ceil((Bmax - sig0)[anyv].max() - b_off)) + 1
            if best is None or W < best[0]:
                best = (W, a2, at, b_off)
        return best

    def seg_cost(self, t0, t1):
        f = self.fit(t0, t1)
        if f is None:
            return 0.0, None
        W, a2, at, b_off = f
        nt = t1 - t0
        # a fitted W in (64,128) pays the sub-512B 2x DMA multiplier on
        # ~512B descriptors; loading junk columns up to W=128 can be cheaper
        best = None
        for Wc in {W, 128} if 64 < W < 128 else {W}:
            fw = nt * Wc
            dma = nt * P * _dma_row_ns(Wc) / 16.0
            # engines: ACT ~1.67/elem (sq+exp), DVE ~1.042 (reduce); treat
            # the binding resource as max(dma, act) + a share of dve
            act = 1.67 * fw + 192 * nt
            dve = 1.042 * fw
            cost = max(dma, act, dve) + 0.25 * (act + dve) + SEG_FIXED_NS
            if best is None or cost < best[0]:
                best = (cost, (Wc, a2, at, b_off))
        return best

    def plan(self, max_segs=6):
        """DP split of [0, ntt) into segments minimizing total cost."""
        nt = self.ntt
        # interval costs
        icost = {}
        ifit = {}
        for t0 in range(nt):
            for t1 in range(t0 + 1, nt + 1):
                c, f = self.seg_cost(t0, t1)
                icost[(t0, t1)] = c
                ifit[(t0, t1)] = f
        INF = float("inf")
        dp = [INF] * (nt + 1)
        prev = [0] * (nt + 1)
        dp[0] = 0.0
        for t1 in range(1, nt + 1):
            for t0 in range(t1):
                c = dp[t0] + icost[(t0, t1)]
                if c < dp[t1]:
                    dp[t1] = c
                    prev[t1] = t0
        cuts = []
        t = nt
        while t > 0:
            t0 = prev[t]
            cuts.append((t0, t))
            t = t0
        cuts.reverse()
        out = []
        for t0, t1 in cuts:
            f = ifit[(t0, t1)]
            if f is None:
                continue
            out.append((t0, t1, f))
        return dp[nt], out

    def quick_cost(self):
        return self.seg_cost(0, self.ntt)[0]


def _build_schedule(input_lengths, output_lengths):
    il = np.asarray(input_lengths, dtype=np.int64)
    ol = np.asarray(output_lengths, dtype=np.int64)
    slopes = il.astype(np.float64) / np.maximum(ol, 1)
    order = np.argsort(slopes, kind="stable")
    groups = [[int(order[8 * g + c]) for c in range(8)] for g in range(8)]

    # hill-climb member swaps on the quick (single-segment) cost
    ctxs = [_ColCtx(gr, il, ol) for gr in groups]
    costs = [c.quick_cost() for c in ctxs]
    rng = np.random.RandomState(0)
    for _ in range(SWAP_ITERS):
        g1, g2 = rng.randint(0, 8, 2)
        if g1 == g2:
            continue
        m1, m2 = rng.randint(0, 8, 2)
        a, bb = groups[g1][m1], groups[g2][m2]
        n1 = groups[g1].copy()
        n2 = groups[g2].copy()
        n1[m1], n2[m2] = bb, a
        c1 = _ColCtx(n1, il, ol)
        c2 = _ColCtx(n2, il, ol)
        if c1.quick_cost() + c2.quick_cost() < costs[g1] + costs[g2] - 1.0:
            groups[g1], groups[g2] = n1, n2
            ctxs[g1], ctxs[g2] = c1, c2
            costs[g1], costs[g2] = c1.quick_cost(), c2.quick_cost()

    assign = [[groups[g][c] for g in range(8)] for c in range(8)]
    segs = []
    for g in range(8):
        ctx = ctxs[g]
        _, plan = ctx.plan()
        for t0, t1, (W, a2, at, b_off) in plan:
            seg = _Seg()
            seg.g = g
            seg.members = groups[g]
            seg.t0 = t0
            seg.nt = t1 - t0
            seg.W = W
            seg.a2 = a2
            seg.at = at
            seg.b = b_off
            rr = np.arange(seg.nt * P)
            seg.sigma = a2 * (rr % P) + at * (rr // P) + b_off
            segs.append(seg)

    # path A/B assignment balancing ACT vs DVE
    act_ns = dve_ns = 0.0
    for seg in sorted(segs, key=lambda s: -s.nt * s.W):
        fw = seg.nt * seg.W
        a_act = 1.67 * fw + 192 * (seg.nt + 1)
        a_dve = 1.042 * fw + 80
        b_act = 1.67 * fw + 384
        b_dve = 2.084 * fw + 245
        if max(act_ns + a_act, dve_ns + a_dve) <= max(
            act_ns + b_act, dve_ns + b_dve
        ):
            seg.path_a = True
            act_ns += a_act
            dve_ns += a_dve
        else:
            seg.path_a = False
            act_ns += b_act
            dve_ns += b_dve

    _coverage_check(segs, il, ol)
    return assign, segs


def _coverage_check(segs, il, ol):
    covered = {}
    for seg in segs:
        rows = seg.nt * P
        i = seg.t0 * P + np.arange(rows)
        for m, b in enumerate(seg.members):
            o, n = int(ol[b]), int(il[b])
            v = i < min(o, T)
            if not v.any():
                continue
            idl = _ideal_f32(i, n, o).astype(np.float64)
            A = np.maximum(0.0, idl - D)
            Bb = np.minimum(n - 1, idl + D)
            ok = (~v) | ((seg.sigma <= A) & (Bb < seg.sigma + seg.W))
            assert ok.all(), (seg.g, b, np.where(~ok)[0][:5])
            base = seg.g * T * E + i * E + seg.sigma
            assert (PADF + base).min() >= 0
            assert (PADF + base + seg.W).max() <= FLAT
            cv = covered.setdefault((seg.g, b), np.zeros(T, bool))
            cv[np.clip(i[v], 0, T - 1)] = True
    # every valid row of every batch must be covered by some segment
    seen = {}
    for seg in segs:
        for b in seg.members:
            seen[(seg.g, b)] = True
    for (g, b) in seen:
        o = min(int(ol[b]), T)
        cv = covered.get((g, b), np.zeros(T, bool))
        assert cv[:o].all(), (g, b, int(np.argmin(cv[:o])))


def _build_nc(segs):
    ntt = sum(s.nt for s in segs)
    nseg = len(segs)
    nc = bacc.Bacc(None, target_bir_lowering=False)
    attn = nc.declare_dram_parameter("attn", [FLAT], F32, isOutput=False)
    center_d = nc.declare_dram_parameter("center", [P, ntt], F32, isOutput=False)
    acc_d = nc.declare_dram_parameter("acc", [P, nseg], F32, isOutput=True)

    with tile.TileContext(nc) as tc:
        with (
            tc.tile_pool(name="const", bufs=1) as const_pool,
            tc.tile_pool(name="at", bufs=3) as at_pool,
            tc.tile_pool(name="d", bufs=3) as d_pool,
            tc.tile_pool(name="g", bufs=3) as g_pool,
            tc.tile_pool(name="junk", bufs=3) as junk_pool,
        ):
            w_i32 = const_pool.tile([P, E], mybir.dt.int32, tag="w_i32")
            w_f32 = const_pool.tile([P, E], F32, tag="w_f32")
            center = const_pool.tile([P, ntt], F32, tag="center")
            acc = const_pool.tile([P, nseg], F32, tag="acc")

            nc.gpsimd.iota(w_i32[:], pattern=[[1, E]], base=0, channel_multiplier=0)
            nc.vector.tensor_copy(w_f32[:], w_i32[:])
            nc.gpsimd.memset(acc[:], 0.0)

            # center table first: every compute op depends on it
            nc.sync.dma_start(out=center[:], in_=center_d[:])
            ats = []
            for seg in segs:
                nt, W = seg.nt, seg.W
                at = at_pool.tile([P, nt * W], F32, tag="at")
                src = AP(
                    attn[:].tensor,
                    PADF + seg.g * T * E + seg.t0 * P * E + seg.b,
                    [[E + seg.a2, P], [P * E + seg.at, nt], [1, W]],
                )
                dst = at[:]
                nc.sync.dma_start(
                    out=AP(dst.tensor, dst.offset, [dst.ap[0], [W, nt], [1, W]]),
                    in_=src,
                )
                ats.append(at)

            k0 = 0
            for si, seg in enumerate(segs):
                nt, W = seg.nt, seg.W
                fw = nt * W
                at = ats[si]
                d2 = junk_pool.tile([P, fw], F32, tag="d2")
                if seg.path_a:
                    for t in range(nt):
                        nc.scalar.activation(
                            d2[:, t * W : (t + 1) * W],
                            w_f32[:, 0:W],
                            AF.Square,
                            bias=center[:, k0 + t : k0 + t + 1],
                            scale=-1.0,
                        )
                else:
                    d = d_pool.tile([P, fw], F32, tag="d")
                    wap = w_f32[:, 0:W]
                    w_b = AP(wap.tensor, wap.offset, [wap.ap[0], [0, nt], [1, W]])
                    cap = center[:, k0 : k0 + nt]
                    c_b = AP(cap.tensor, cap.offset, [cap.ap[0], [1, nt], [0, W]])
                    dap = d[:]
                    d3 = AP(dap.tensor, dap.offset, [dap.ap[0], [W, nt], [1, W]])
                    nc.vector.tensor_tensor(d3, w_b, c_b, OP.subtract)
                    nc.scalar.activation(d2[:], d[:], AF.Square)
                gt = g_pool.tile([P, fw], F32, tag="gt")
                nc.scalar.activation(gt[:], d2[:], AF.Exp, scale=NEG_SCALE)
                jk = junk_pool.tile([P, fw], F32, tag="jk")
                nc.vector.scalar_tensor_tensor(
                    jk[:], gt[:], 1.0, at[:], OP.mult, OP.mult,
                    accum_out=acc[:, si : si + 1],
                )
                k0 += nt
            nc.sync.dma_start(out=acc_d[:], in_=acc[:])
    return nc


def _get_nc(segs):
    key = tuple(s.key() for s in segs)
    if key not in _NC_CACHE:
        nc = _build_nc(segs)
        if not nc.is_finalized():
            nc.finalize()
        _NC_CACHE[key] = nc
    return _NC_CACHE[key]


def _make_tables(il, ol, assign_c, segs):
    ntt = sum(s.nt for s in segs)
    center = np.full((P, ntt), 1e4, np.float32)
    k0 = 0
    for seg in segs:
        b = assign_c[seg.g]
        o, n = int(ol[b]), int(il[b])
        rows = seg.nt * P
        i = seg.t0 * P + np.arange(rows)
        idl = _ideal_f32(i, n, o)
        validr = i < min(o, T)
        cen = np.where(validr, idl - seg.sigma.astype(np.float32), np.float32(1e4))
        center[:, k0 : k0 + seg.nt] = cen.reshape(seg.nt, P).T
        k0 += seg.nt
    return {"center": center}


def _garbage_correction(in_maps, il, ol, assign, segs):
    """Sum of out-of-range contributions the device wrongly included."""
    M = 24
    corr = 0.0
    for c in range(N_CORES):
        flat = in_maps[c]["attn"]
        for seg in segs:
            b = assign[c][seg.g]
            o, n = int(ol[b]), int(il[b])
            lim = min(n, E)
            rows = seg.nt * P
            i = seg.t0 * P + np.arange(rows)
            validr = i < min(o, T)
            idl = _ideal_f32(i, n, o).astype(np.float64)
            sg = seg.sigma
            fr = validr & (
                ((sg < 0) & (idl <= M)) | ((sg + seg.W > lim) & (idl >= lim - M))
            )
            if not fr.any():
                continue
            ii = i[fr]
            j = sg[fr][:, None] + np.arange(seg.W)[None, :]
            dd = j - idl[fr][:, None]
            bad = ((j < 0) | (j >= lim)) & (np.abs(dd) <= M)
            if not bad.any():
                continue
            addr = PADF + seg.g * T * E + ii[:, None] * E + j
            vals = flat[addr[bad]].astype(np.float64)
            corr += float(np.sum(np.exp(-3.125 * dd[bad] ** 2) * vals))
    return corr


def _run(attention_weights, input_lengths, output_lengths, **spmd_kwargs):
    attention_weights = np.ascontiguousarray(attention_weights, dtype=np.float32)
    il = np.asarray(input_lengths, dtype=np.int64)
    ol = np.asarray(output_lengths, dtype=np.int64)
    assign, segs = _build_schedule(il, ol)
    in_maps = []
    for c in range(N_CORES):
        flat = np.empty(FLAT, np.float32)
        flat[:PADF] = 0.0
        flat[PADF : PADF + B_LOC * T * E] = attention_weights[assign[c]].reshape(-1)
        flat[PADF + B_LOC * T * E :] = 0.0
        in_maps.append({"attn": flat, **_make_tables(il, ol, assign[c], segs)})
    res = run_bass_kernel_spmd(
        _get_nc(segs), in_maps, list(range(N_CORES)), **spmd_kwargs
    )
    total = sum(float(r["acc"].sum(dtype=np.float64)) for r in res.results)
    total -= _garbage_correction(in_maps, il, ol, assign, segs)
    return np.float32(total / float(B * T * E)), res


def kernel(attention_weights, input_lengths, output_lengths):
    out, _ = _run(attention_weights, input_lengths, output_lengths)
    return out


# revision 29
# speedup vs baseline: 1.5762x; 1.2926x over previous
"""GuidedAttentionLoss on Trainium2 — 8 NeuronCores, per-core-specialized
diagonal-band gather kernels.

loss = mean(attention_weights * mask), mask[b,i,j] =
    (i < out_len_b) & (j < in_len_b) ? exp(-(j - floor(i/out*in))^2 / (2*0.4^2)) : 0

With sigma=0.4 the Gaussian underflows to exactly 0 in f32 beyond
|j - ideal_i| ~ 4.6, so per valid row only a ~9-wide band of columns
contributes. Each core gets 8 whole batches (greedy-balanced by cost) and its
OWN compiled program specialized to them: per batch a quantized shear line
sigma(i) = a2*(i%128) + at*(i//128) + b tracks ideal(i), and a single 3-dim
DMA access pattern [[400+a2,128],[51200+at,nt],[1,W]] gathers the whole
batch's band ([128 rows/tile] x [W cols], nt tiles) in ONE DMA instruction,
with W fitted exactly (rows past out_len get center=+1e4 and die in the
exp underflow). Per segment:
    path A: d2[:, t] = ACT Square(-w + center_t)   (per-tile bias, no sub)
    path B: d = w - center (DVE, broadcast APs); d2 = ACT Square(d)
    g = ACT Exp(-3.125*d2);  acc[:,s] += g*attn  (DVE stt accum)
Paths are chosen per segment to balance ACT vs DVE. Out-of-range garbage
(front spill j<0, j >= min(in,400)) is not masked on device; the host
subtracts those few boundary terms exactly in f64.

The 8 programs run concurrently: each is jit-compiled for its own device and
dispatched asynchronously; results are gathered and summed on host.
"""

import numpy as np

import concourse.bacc as bacc
import concourse.bass as bass  # noqa: F401
import concourse.mybir as mybir
from concourse.ap import AP
from concourse import tile

N_CORES = 8
B, T, E = 64, 2000, 400
B_LOC = B // N_CORES
P = 128
D = 4
PADF = 512
PADB = 81920
FLAT = PADF + B_LOC * T * E + PADB
NEG_SCALE = -3.125
F32 = mybir.dt.float32
AF = mybir.ActivationFunctionType
OP = mybir.AluOpType

SEG_FIXED_NS = 900.0

_PLAN_CACHE = {}
_EXEC_CACHE = {}


def _ideal_f32(i, in_len, out_len):
    safe_out = np.float32(max(float(out_len), 1.0))
    return np.floor((i.astype(np.float32) / safe_out) * np.float32(in_len)).astype(
        np.float32
    )


def _dma_row_ns(W):
    by = 4 * W
    mult = 2.0 if by < 512 else 1.0
    return max(by * mult / 22.5, 7.0)


class _Seg:
    __slots__ = ("g", "t0", "nt", "W", "a2", "at", "b", "sigma", "path_a")

    def key(self):
        return (self.g, self.t0, self.nt, self.W, self.a2, self.at, self.b,
                self.path_a)


class _BatchCtx:
    """Row band data for a single batch."""

    def __init__(self, b, il, ol):
        self.b = b
        o, n = int(ol[b]), int(il[b])
        self.out = min(o, T)
        self.ntt = (self.out + P - 1) // P
        rows = self.ntt * P
        i = np.arange(rows)
        self.valid = i < self.out
        idl = _ideal_f32(i, n, o).astype(np.float64)
        self.A = np.maximum(0.0, idl - D)
        self.Bb = np.minimum(n - 1, idl + D)
        self.slope = il[b] / max(ol[b], 1)

    def fit(self, t0, t1):
        rows = (t1 - t0) * P
        sl = slice(t0 * P, t1 * P)
        anyv = self.valid[sl]
        if not anyv.any():
            return None
        Amin = self.A[sl]
        Bmax = self.Bb[sl]
        rr = np.arange(rows)
        t_idx = rr // P
        p = rr % P
        s = self.slope
        cands = set()
        for f1 in (np.floor, np.round):
            a2 = int(f1(s))
            for f3 in (np.floor, np.round):
                at3 = int(f3(128 * s))
                for dat in (-1, 0, 1):
                    cands.add((a2, at3 + dat))
        cands.add((0, 0))
        best = None
        for a2, at in cands:
            sig0 = a2 * p + at * t_idx
            b_off = int(np.floor((Amin - sig0)[anyv].min()))
            W = int(np.ceil((Bmax - sig0)[anyv].max() - b_off)) + 1
            if best is None or W < best[0]:
                best = (W, a2, at, b_off)
        return best

    def seg_cost(self, t0, t1):
        f = self.fit(t0, t1)
        if f is None:
            return 0.0, None
        W, a2, at, b_off = f
        nt = t1 - t0
        best = None
        for Wc in {W, 128} if 64 < W < 128 else {W}:
            fw = nt * Wc
            dma = nt * P * _dma_row_ns(Wc) / 16.0
            act = 1.67 * fw + 192 * nt
            dve = 1.042 * fw
            cost = max(dma, act, dve) + 0.25 * (act + dve) + SEG_FIXED_NS
            if best is None or cost < best[0]:
                best = (cost, (Wc, a2, at, b_off))
        return best

    def plan(self):
        nt = self.ntt
        icost = {}
        ifit = {}
        for t0 in range(nt):
            for t1 in range(t0 + 1, nt + 1):
                c, f = self.seg_cost(t0, t1)
                icost[(t0, t1)] = c
                ifit[(t0, t1)] = f
        INF = float("inf")
        dp = [INF] * (nt + 1)
        prev = [0] * (nt + 1)
        dp[0] = 0.0
        for t1 in range(1, nt + 1):
            for t0 in range(t1):
                c = dp[t0] + icost[(t0, t1)]
                if c < dp[t1]:
                    dp[t1] = c
                    prev[t1] = t0
        cuts = []
        t = nt
        while t > 0:
            t0 = prev[t]
            cuts.append((t0, t))
            t = t0
        cuts.reverse()
        out = [(t0, t1, ifit[(t0, t1)]) for t0, t1 in cuts if ifit[(t0, t1)]]
        return dp[nt], out


def _build_schedule(input_lengths, output_lengths):
    """Returns assign[c][g] = batch at core c slot g, and per-core seg lists."""
    il = np.asarray(input_lengths, dtype=np.int64)
    ol = np.asarray(output_lengths, dtype=np.int64)
    ctxs = [_BatchCtx(b, il, ol) for b in range(B)]
    plans = [c.plan() for c in ctxs]
    order = sorted(range(B), key=lambda b: -plans[b][0])
    loads = [0.0] * N_CORES
    slots = [[] for _ in range(N_CORES)]
    for b in order:
        c = min(
            (c for c in range(N_CORES) if len(slots[c]) < B_LOC),
            key=lambda c: loads[c],
        )
        slots[c].append(b)
        loads[c] += plans[b][0]
    assign = [slots[c] for c in range(N_CORES)]

    core_segs = []
    for c in range(N_CORES):
        segs = []
        for g, b in enumerate(assign[c]):
            for t0, t1, (W, a2, at, b_off) in plans[b][1]:
                seg = _Seg()
                seg.g = g
                seg.t0 = t0
                seg.nt = t1 - t0
                seg.W = W
                seg.a2 = a2
                seg.at = at
                seg.b = b_off
                rr = np.arange(seg.nt * P)
                seg.sigma = a2 * (rr % P) + at * (rr // P) + b_off
                segs.append(seg)
        # path A/B to balance ACT vs DVE within this core
        act_ns = dve_ns = 0.0
        for seg in sorted(segs, key=lambda s: -s.nt * s.W):
            fw = seg.nt * seg.W
            a_act = 1.67 * fw + 192 * (seg.nt + 1)
            a_dve = 1.042 * fw + 80
            b_act = 1.67 * fw + 384
            b_dve = 2.084 * fw + 245
            if max(act_ns + a_act, dve_ns + a_dve) <= max(
                act_ns + b_act, dve_ns + b_dve
            ):
                seg.path_a = True
                act_ns += a_act
                dve_ns += a_dve
            else:
                seg.path_a = False
                act_ns += b_act
                dve_ns += b_dve
        _coverage_check(segs, assign[c], il, ol)
        core_segs.append(segs)
    return assign, core_segs


def _coverage_check(segs, assign_c, il, ol):
    covered = {}
    for seg in segs:
        b = assign_c[seg.g]
        o, n = int(ol[b]), int(il[b])
        rows = seg.nt * P
        i = seg.t0 * P + np.arange(rows)
        v = i < min(o, T)
        if v.any():
            idl = _ideal_f32(i, n, o).astype(np.float64)
            A = np.maximum(0.0, idl - D)
            Bb = np.minimum(n - 1, idl + D)
            ok = (~v) | ((seg.sigma <= A) & (Bb < seg.sigma + seg.W))
            assert ok.all(), (seg.g, b, np.where(~ok)[0][:5])
        base = seg.g * T * E + i * E + seg.sigma
        assert (PADF + base).min() >= 0
        assert (PADF + base + seg.W).max() <= FLAT
        cv = covered.setdefault(seg.g, np.zeros(T, bool))
        cv[np.clip(i[v], 0, T - 1)] = True
    for g, b in enumerate(assign_c):
        o = min(int(ol[b]), T)
        cv = covered.get(g, np.zeros(T, bool))
        assert cv[:o].all(), (g, b, int(np.argmin(cv[:o])))


def _build_nc(segs):
    ntt = sum(s.nt for s in segs)
    nseg = len(segs)
    nc = bacc.Bacc(None, target_bir_lowering=False)
    attn = nc.declare_dram_parameter("attn", [FLAT], F32, isOutput=False)
    center_d = nc.declare_dram_parameter("center", [P, ntt], F32, isOutput=False)
    acc_d = nc.declare_dram_parameter("acc", [P, nseg], F32, isOutput=True)

    with tile.TileContext(nc) as tc:
        with (
            tc.tile_pool(name="const", bufs=1) as const_pool,
            tc.tile_pool(name="at", bufs=3) as at_pool,
            tc.tile_pool(name="d", bufs=3) as d_pool,
            tc.tile_pool(name="g", bufs=3) as g_pool,
            tc.tile_pool(name="junk", bufs=3) as junk_pool,
        ):
            w_i32 = const_pool.tile([P, E], mybir.dt.int32, tag="w_i32")
            w_f32 = const_pool.tile([P, E], F32, tag="w_f32")
            center = const_pool.tile([P, ntt], F32, tag="center")
            acc = const_pool.tile([P, nseg], F32, tag="acc")

            nc.gpsimd.iota(w_i32[:], pattern=[[1, E]], base=0, channel_multiplier=0)
            nc.vector.tensor_copy(w_f32[:], w_i32[:])
            nc.gpsimd.memset(acc[:], 0.0)

            # center table first: every compute op depends on it
            nc.sync.dma_start(out=center[:], in_=center_d[:])
            ats = []
            for seg in segs:
                nt, W = seg.nt, seg.W
                at = at_pool.tile([P, nt * W], F32, tag="at")
                src = AP(
                    attn[:].tensor,
                    PADF + seg.g * T * E + seg.t0 * P * E + seg.b,
                    [[E + seg.a2, P], [P * E + seg.at, nt], [1, W]],
                )
                dst = at[:]
                nc.sync.dma_start(
                    out=AP(dst.tensor, dst.offset, [dst.ap[0], [W, nt], [1, W]]),
                    in_=src,
                )
                ats.append(at)

            k0 = 0
            for si, seg in enumerate(segs):
                nt, W = seg.nt, seg.W
                fw = nt * W
                at = ats[si]
                d2 = junk_pool.tile([P, fw], F32, tag="d2")
                if seg.path_a:
                    for t in range(nt):
                        nc.scalar.activation(
                            d2[:, t * W : (t + 1) * W],
                            w_f32[:, 0:W],
                            AF.Square,
                            bias=center[:, k0 + t : k0 + t + 1],
                            scale=-1.0,
                        )
                else:
                    d = d_pool.tile([P, fw], F32, tag="d")
                    wap = w_f32[:, 0:W]
                    w_b = AP(wap.tensor, wap.offset, [wap.ap[0], [0, nt], [1, W]])
                    cap = center[:, k0 : k0 + nt]
                    c_b = AP(cap.tensor, cap.offset, [cap.ap[0], [1, nt], [0, W]])
                    dap = d[:]
                    d3 = AP(dap.tensor, dap.offset, [dap.ap[0], [W, nt], [1, W]])
                    nc.vector.tensor_tensor(d3, w_b, c_b, OP.subtract)
                    nc.scalar.activation(d2[:], d[:], AF.Square)
                gt = g_pool.tile([P, fw], F32, tag="gt")
                nc.scalar.activation(gt[:], d2[:], AF.Exp, scale=NEG_SCALE)
                jk = junk_pool.tile([P, fw], F32, tag="jk")
                nc.vector.scalar_tensor_tensor(
                    jk[:], gt[:], 1.0, at[:], OP.mult, OP.mult,
                    accum_out=acc[:, si : si + 1],
                )
                k0 += nt
            nc.sync.dma_start(out=acc_d[:], in_=acc[:])
    return nc


def _make_tables(il, ol, assign_c, segs):
    ntt = sum(s.nt for s in segs)
    center = np.full((P, ntt), 1e4, np.float32)
    k0 = 0
    for seg in segs:
        b = assign_c[seg.g]
        o, n = int(ol[b]), int(il[b])
        rows = seg.nt * P
        i = seg.t0 * P + np.arange(rows)
        idl = _ideal_f32(i, n, o)
        validr = i < min(o, T)
        cen = np.where(validr, idl - seg.sigma.astype(np.float32), np.float32(1e4))
        center[:, k0 : k0 + seg.nt] = cen.reshape(seg.nt, P).T
        k0 += seg.nt
    return {"center": center}


def _garbage_correction(in_maps, il, ol, assign, core_segs):
    M = 24
    corr = 0.0
    for c in range(N_CORES):
        flat = in_maps[c]["attn"]
        for seg in core_segs[c]:
            b = assign[c][seg.g]
            o, n = int(ol[b]), int(il[b])
            lim = min(n, E)
            rows = seg.nt * P
            i = seg.t0 * P + np.arange(rows)
            validr = i < min(o, T)
            idl = _ideal_f32(i, n, o).astype(np.float64)
            sg = seg.sigma
            fr = validr & (
                ((sg < 0) & (idl <= M)) | ((sg + seg.W > lim) & (idl >= lim - M))
            )
            if not fr.any():
                continue
            ii = i[fr]
            j = sg[fr][:, None] + np.arange(seg.W)[None, :]
            dd = j - idl[fr][:, None]
            bad = ((j < 0) | (j >= lim)) & (np.abs(dd) <= M)
            if not bad.any():
                continue
            addr = PADF + seg.g * T * E + ii[:, None] * E + j
            vals = flat[addr[bad]].astype(np.float64)
            corr += float(np.sum(np.exp(-3.125 * dd[bad] ** 2) * vals))
    return corr


def _get_compiled(c, segs, in_map):
    """jit-compile core c's program for device c; cache across calls."""
    import jax
    from concourse import bass2jax
    from concourse.bass2jax import _bass_exec_p

    key = (c, tuple(s.key() for s in segs))
    if key in _EXEC_CACHE:
        return _EXEC_CACHE[key]

    bass2jax.install_neuronx_cc_hook()
    nc = _build_nc(segs)
    if not nc.is_finalized():
        nc.finalize()

    in_names, out_names, out_avals, zero_outs = [], [], [], []
    for alloc in nc.m.functions[0].allocations:
        if not isinstance(alloc, mybir.MemoryLocationSet):
            continue
        name = alloc.memorylocations[0].name
        if alloc.kind == "ExternalInput":
            in_names.append(name)
        elif alloc.kind == "ExternalOutput":
            out_names.append(name)
            shape = tuple(alloc.tensor_shape)
            dtype = mybir.dt.np(alloc.dtype)
            out_avals.append(jax.core.ShapedArray(shape, dtype))
            zero_outs.append(np.zeros(shape, dtype))
    n_params = len(in_names)
    all_names = in_names + out_names
    donate = tuple(range(n_params, n_params + len(out_names)))

    def _body(*args):
        outs = _bass_exec_p.bind(
            *args,
            out_avals=tuple(out_avals),
            in_names=tuple(all_names),
            out_names=tuple(out_names),
            lowering_input_output_aliases=(),
            sim_require_finite=True,
            sim_require_nnan=True,
            nc=nc,
        )
        return tuple(outs)

    dev = jax.devices()[c]
    with jax.default_device(dev):
        jf = jax.jit(_body, donate_argnums=donate, keep_unused=True)
        args = _core_args(nc, in_names, zero_outs, in_map, c)
        comp = jf.lower(*args).compile()
    entry = (comp, nc, in_names, out_names, zero_outs)
    _EXEC_CACHE[key] = entry
    return entry


def _core_args(nc, in_names, zero_outs, in_map, c):
    im = dict(in_map)
    if nc.partition_id_tensor is not None:
        im[nc.partition_id_tensor.name] = np.array([[c]], dtype=np.uint32)
    return [np.asarray(im[n]) for n in in_names] + [z.copy() for z in zero_outs]


def _run(attention_weights, input_lengths, output_lengths, ntff_hook=None):
    attention_weights = np.ascontiguousarray(attention_weights, dtype=np.float32)
    il = np.asarray(input_lengths, dtype=np.int64)
    ol = np.asarray(output_lengths, dtype=np.int64)
    assign, core_segs = _build_schedule(il, ol)
    in_maps = []
    for c in range(N_CORES):
        flat = np.empty(FLAT, np.float32)
        flat[:PADF] = 0.0
        flat[PADF : PADF + B_LOC * T * E] = attention_weights[assign[c]].reshape(-1)
        flat[PADF + B_LOC * T * E :] = 0.0
        in_maps.append(
            {"attn": flat, **_make_tables(il, ol, assign[c], core_segs[c])}
        )

    entries = [
        _get_compiled(c, core_segs[c], in_maps[c]) for c in range(N_CORES)
    ]

    def _dispatch():
        futs = []
        for c, (comp, nc, in_names, out_names, zero_outs) in enumerate(entries):
            args = _core_args(nc, in_names, zero_outs, in_maps[c], c)
            futs.append((comp(*args), out_names))
        return [
            {name: np.asarray(v) for name, v in zip(out_names, outs)}
            for outs, out_names in futs
        ]

    if ntff_hook is not None:
        with ntff_hook:
            results = _dispatch()
    else:
        results = _dispatch()

    total = sum(float(r["acc"].sum(dtype=np.float64)) for r in results)
    total -= _garbage_correction(in_maps, il, ol, assign, core_segs)
    return np.float32(total / float(B * T * E)), results


def kernel(attention_weights, input_lengths, output_lengths):
    out, _ = _run(attention_weights, input_lengths, output_lengths)
    return out


# revision 30
# speedup vs baseline: 2.0894x; 1.3256x over previous
"""GuidedAttentionLoss on Trainium2 — 8 NeuronCores, per-core-specialized
diagonal-band gather kernels.

loss = mean(attention_weights * mask), mask[b,i,j] =
    (i < out_len_b) & (j < in_len_b) ? exp(-(j - floor(i/out*in))^2 / (2*0.4^2)) : 0

With sigma=0.4 the Gaussian underflows to exactly 0 in f32 beyond
|j - ideal_i| ~ 4.6, so per valid row only a ~9-wide band of columns
contributes. Each core gets 8 whole batches (greedy-balanced by cost) and its
OWN compiled program specialized to them: per batch a quantized shear line
sigma(i) = a2*(i%128) + at*(i//128) + b tracks ideal(i), and a single 3-dim
DMA access pattern [[400+a2,128],[51200+at,nt],[1,W]] gathers the whole
batch's band ([128 rows/tile] x [W cols], nt tiles) in ONE DMA instruction,
with W fitted exactly (rows past out_len get center=+1e4 and die in the
exp underflow). Per segment:
    path A: d2[:, t] = ACT Square(-w + center_t)   (per-tile bias, no sub)
    path B: d = w - center (DVE, broadcast APs); d2 = ACT Square(d)
    g = ACT Exp(-3.125*d2);  acc[:,s] += g*attn  (DVE stt accum)
Paths are chosen per segment to balance ACT vs DVE. Out-of-range garbage
(front spill j<0, j >= min(in,400)) is not masked on device; the host
subtracts those few boundary terms exactly in f64.

The 8 programs run concurrently: each is jit-compiled for its own device and
dispatched asynchronously; results are gathered and summed on host.
"""

import numpy as np

import concourse.bacc as bacc
import concourse.bass as bass  # noqa: F401
import concourse.mybir as mybir
from concourse.ap import AP
from concourse import tile

N_CORES = 8
B, T, E = 64, 2000, 400
B_LOC = B // N_CORES
P = 128
D = 4
PADF = 512
PADB = 81920
FLAT = PADF + B_LOC * T * E + PADB
NEG_SCALE = -3.125
F32 = mybir.dt.float32
AF = mybir.ActivationFunctionType
OP = mybir.AluOpType

SEG_FIXED_NS = 900.0

_PLAN_CACHE = {}
_EXEC_CACHE = {}


def _ideal_f32(i, in_len, out_len):
    safe_out = np.float32(max(float(out_len), 1.0))
    return np.floor((i.astype(np.float32) / safe_out) * np.float32(in_len)).astype(
        np.float32
    )


def _dma_row_ns(W):
    by = 4 * W
    mult = 2.0 if by < 512 else 1.0
    return max(by * mult / 22.5, 7.0)


class _Seg:
    __slots__ = ("g", "t0", "nt", "W", "a2", "at", "b", "sigma", "path_a")

    def key(self):
        return (self.g, self.t0, self.nt, self.W, self.a2, self.at, self.b,
                self.path_a)


class _BatchCtx:
    """Row band data for a single batch."""

    def __init__(self, b, il, ol):
        self.b = b
        o, n = int(ol[b]), int(il[b])
        self.out = min(o, T)
        self.ntt = (self.out + P - 1) // P
        rows = self.ntt * P
        i = np.arange(rows)
        self.valid = i < self.out
        idl = _ideal_f32(i, n, o).astype(np.float64)
        self.A = np.maximum(0.0, idl - D)
        self.Bb = np.minimum(n - 1, idl + D)
        self.slope = il[b] / max(ol[b], 1)

    def fit(self, t0, t1):
        rows = (t1 - t0) * P
        sl = slice(t0 * P, t1 * P)
        anyv = self.valid[sl]
        if not anyv.any():
            return None
        Amin = self.A[sl]
        Bmax = self.Bb[sl]
        rr = np.arange(rows)
        t_idx = rr // P
        p = rr % P
        s = self.slope
        cands = set()
        for f1 in (np.floor, np.round):
            a2 = int(f1(s))
            for f3 in (np.floor, np.round):
                at3 = int(f3(128 * s))
                for dat in (-1, 0, 1):
                    cands.add((a2, at3 + dat))
        cands.add((0, 0))
        best = None
        for a2, at in cands:
            sig0 = a2 * p + at * t_idx
            b_off = int(np.floor((Amin - sig0)[anyv].min()))
            W = int(np.ceil((Bmax - sig0)[anyv].max() - b_off)) + 1
            if best is None or W < best[0]:
                best = (W, a2, at, b_off)
        return best

    def seg_cost(self, t0, t1):
        f = self.fit(t0, t1)
        if f is None:
            return 0.0, None
        W, a2, at, b_off = f
        nt = t1 - t0
        best = None
        for Wc in {W, 128} if 64 < W < 128 else {W}:
            fw = nt * Wc
            dma = nt * P * _dma_row_ns(Wc) / 16.0
            act = 1.67 * fw + 192 * nt
            dve = 1.042 * fw
            cost = max(dma, act, dve) + 0.25 * (act + dve) + SEG_FIXED_NS
            if best is None or cost < best[0]:
                best = (cost, (Wc, a2, at, b_off))
        return best

    def plan(self):
        nt = self.ntt
        icost = {}
        ifit = {}
        for t0 in range(nt):
            for t1 in range(t0 + 1, nt + 1):
                c, f = self.seg_cost(t0, t1)
                icost[(t0, t1)] = c
                ifit[(t0, t1)] = f
        INF = float("inf")
        dp = [INF] * (nt + 1)
        prev = [0] * (nt + 1)
        dp[0] = 0.0
        for t1 in range(1, nt + 1):
            for t0 in range(t1):
                c = dp[t0] + icost[(t0, t1)]
                if c < dp[t1]:
                    dp[t1] = c
                    prev[t1] = t0
        cuts = []
        t = nt
        while t > 0:
            t0 = prev[t]
            cuts.append((t0, t))
            t = t0
        cuts.reverse()
        out = [(t0, t1, ifit[(t0, t1)]) for t0, t1 in cuts if ifit[(t0, t1)]]
        return dp[nt], out


def _build_schedule(input_lengths, output_lengths):
    """Returns assign[c][g] = batch at core c slot g, and per-core seg lists."""
    il = np.asarray(input_lengths, dtype=np.int64)
    ol = np.asarray(output_lengths, dtype=np.int64)
    ctxs = [_BatchCtx(b, il, ol) for b in range(B)]
    plans = [c.plan() for c in ctxs]
    order = sorted(range(B), key=lambda b: -plans[b][0])
    loads = [0.0] * N_CORES
    slots = [[] for _ in range(N_CORES)]
    for b in order:
        c = min(
            (c for c in range(N_CORES) if len(slots[c]) < B_LOC),
            key=lambda c: loads[c],
        )
        slots[c].append(b)
        loads[c] += plans[b][0]
    assign = [slots[c] for c in range(N_CORES)]

    core_segs = []
    for c in range(N_CORES):
        segs = []
        for g, b in enumerate(assign[c]):
            for t0, t1, (W, a2, at, b_off) in plans[b][1]:
                seg = _Seg()
                seg.g = g
                seg.t0 = t0
                seg.nt = t1 - t0
                seg.W = W
                seg.a2 = a2
                seg.at = at
                seg.b = b_off
                rr = np.arange(seg.nt * P)
                seg.sigma = a2 * (rr % P) + at * (rr // P) + b_off
                segs.append(seg)
        # path A/B to balance ACT vs DVE within this core
        act_ns = dve_ns = 0.0
        for seg in sorted(segs, key=lambda s: -s.nt * s.W):
            fw = seg.nt * seg.W
            a_act = 1.67 * fw + 192 * (seg.nt + 1)
            a_dve = 1.042 * fw + 80
            b_act = 1.67 * fw + 384
            b_dve = 2.084 * fw + 245
            if max(act_ns + a_act, dve_ns + a_dve) <= max(
                act_ns + b_act, dve_ns + b_dve
            ):
                seg.path_a = True
                act_ns += a_act
                dve_ns += a_dve
            else:
                seg.path_a = False
                act_ns += b_act
                dve_ns += b_dve
        _coverage_check(segs, assign[c], il, ol)
        core_segs.append(segs)
    return assign, core_segs


def _coverage_check(segs, assign_c, il, ol):
    covered = {}
    for seg in segs:
        b = assign_c[seg.g]
        o, n = int(ol[b]), int(il[b])
        rows = seg.nt * P
        i = seg.t0 * P + np.arange(rows)
        v = i < min(o, T)
        if v.any():
            idl = _ideal_f32(i, n, o).astype(np.float64)
            A = np.maximum(0.0, idl - D)
            Bb = np.minimum(n - 1, idl + D)
            ok = (~v) | ((seg.sigma <= A) & (Bb < seg.sigma + seg.W))
            assert ok.all(), (seg.g, b, np.where(~ok)[0][:5])
        base = seg.g * T * E + i * E + seg.sigma
        assert (PADF + base).min() >= 0
        assert (PADF + base + seg.W).max() <= FLAT
        cv = covered.setdefault(seg.g, np.zeros(T, bool))
        cv[np.clip(i[v], 0, T - 1)] = True
    for g, b in enumerate(assign_c):
        o = min(int(ol[b]), T)
        cv = covered.get(g, np.zeros(T, bool))
        assert cv[:o].all(), (g, b, int(np.argmin(cv[:o])))


def _build_nc(segs):
    ntt = sum(s.nt for s in segs)
    nseg = len(segs)
    nc = bacc.Bacc(None, target_bir_lowering=False)
    attn = nc.declare_dram_parameter("attn", [FLAT], F32, isOutput=False)
    center_d = nc.declare_dram_parameter("center", [P, ntt], F32, isOutput=False)
    acc_d = nc.declare_dram_parameter("acc", [P, nseg], F32, isOutput=True)

    with tile.TileContext(nc) as tc:
        with (
            tc.tile_pool(name="const", bufs=1) as const_pool,
            tc.tile_pool(name="at", bufs=3) as at_pool,
            tc.tile_pool(name="d", bufs=3) as d_pool,
            tc.tile_pool(name="g", bufs=3) as g_pool,
            tc.tile_pool(name="junk", bufs=3) as junk_pool,
        ):
            w_i32 = const_pool.tile([P, E], mybir.dt.int32, tag="w_i32")
            w_f32 = const_pool.tile([P, E], F32, tag="w_f32")
            center = const_pool.tile([P, ntt], F32, tag="center")
            acc = const_pool.tile([P, nseg], F32, tag="acc")

            nc.gpsimd.iota(w_i32[:], pattern=[[1, E]], base=0, channel_multiplier=0)
            nc.vector.tensor_copy(w_f32[:], w_i32[:])
            nc.gpsimd.memset(acc[:], 0.0)

            # center table first: every compute op depends on it
            nc.sync.dma_start(out=center[:], in_=center_d[:])

            # group segments into chunks; one Square/Exp/reduce per chunk
            chunks = []
            cur = []
            cfw = 0
            for si, seg in enumerate(segs):
                cur.append(si)
                cfw += seg.nt * seg.W
                if cfw >= 640:
                    chunks.append(cur)
                    cur = []
                    cfw = 0
            if cur:
                chunks.append(cur)

            k0s = np.cumsum([0] + [s.nt for s in segs])
            for ci, chunk in enumerate(chunks):
                fwc = sum(segs[si].nt * segs[si].W for si in chunk)
                at = at_pool.tile([P, fwc], F32, tag="at")
                d = d_pool.tile([P, fwc], F32, tag="d")
                off = 0
                for si in chunk:
                    seg = segs[si]
                    nt, W = seg.nt, seg.W
                    src = AP(
                        attn[:].tensor,
                        PADF + seg.g * T * E + seg.t0 * P * E + seg.b,
                        [[E + seg.a2, P], [P * E + seg.at, nt], [1, W]],
                    )
                    dst = at[:, off : off + nt * W]
                    nc.sync.dma_start(
                        out=AP(dst.tensor, dst.offset,
                               [dst.ap[0], [W, nt], [1, W]]),
                        in_=src,
                    )
                    # d = w - center (broadcast APs)
                    k0 = int(k0s[si])
                    wap = w_f32[:, 0:W]
                    w_b = AP(wap.tensor, wap.offset,
                             [wap.ap[0], [0, nt], [1, W]])
                    cap = center[:, k0 : k0 + nt]
                    c_b = AP(cap.tensor, cap.offset,
                             [cap.ap[0], [1, nt], [0, W]])
                    dsl = d[:, off : off + nt * W]
                    d3 = AP(dsl.tensor, dsl.offset,
                            [dsl.ap[0], [W, nt], [1, W]])
                    nc.vector.tensor_tensor(d3, w_b, c_b, OP.subtract)
                    off += nt * W
                d2 = junk_pool.tile([P, fwc], F32, tag="d2")
                nc.scalar.activation(d2[:], d[:], AF.Square)
                gt = g_pool.tile([P, fwc], F32, tag="gt")
                nc.scalar.activation(gt[:], d2[:], AF.Exp, scale=NEG_SCALE)
                jk = junk_pool.tile([P, fwc], F32, tag="jk")
                nc.vector.scalar_tensor_tensor(
                    jk[:], gt[:], 1.0, at[:], OP.mult, OP.mult,
                    accum_out=acc[:, ci : ci + 1],
                )
            nc.sync.dma_start(out=acc_d[:], in_=acc[:])
    return nc


def _make_tables(il, ol, assign_c, segs):
    ntt = sum(s.nt for s in segs)
    center = np.full((P, ntt), 1e4, np.float32)
    k0 = 0
    for seg in segs:
        b = assign_c[seg.g]
        o, n = int(ol[b]), int(il[b])
        rows = seg.nt * P
        i = seg.t0 * P + np.arange(rows)
        idl = _ideal_f32(i, n, o)
        validr = i < min(o, T)
        cen = np.where(validr, idl - seg.sigma.astype(np.float32), np.float32(1e4))
        center[:, k0 : k0 + seg.nt] = cen.reshape(seg.nt, P).T
        k0 += seg.nt
    return {"center": center}


def _garbage_correction(in_maps, il, ol, assign, core_segs):
    M = 24
    corr = 0.0
    for c in range(N_CORES):
        flat = in_maps[c]["attn"]
        for seg in core_segs[c]:
            b = assign[c][seg.g]
            o, n = int(ol[b]), int(il[b])
            lim = min(n, E)
            rows = seg.nt * P
            i = seg.t0 * P + np.arange(rows)
            validr = i < min(o, T)
            idl = _ideal_f32(i, n, o).astype(np.float64)
            sg = seg.sigma
            fr = validr & (
                ((sg < 0) & (idl <= M)) | ((sg + seg.W > lim) & (idl >= lim - M))
            )
            if not fr.any():
                continue
            ii = i[fr]
            j = sg[fr][:, None] + np.arange(seg.W)[None, :]
            dd = j - idl[fr][:, None]
            bad = ((j < 0) | (j >= lim)) & (np.abs(dd) <= M)
            if not bad.any():
                continue
            addr = PADF + seg.g * T * E + ii[:, None] * E + j
            vals = flat[addr[bad]].astype(np.float64)
            corr += float(np.sum(np.exp(-3.125 * dd[bad] ** 2) * vals))
    return corr


def _get_compiled(c, segs, in_map):
    """jit-compile core c's program for device c; cache across calls."""
    import jax
    from concourse import bass2jax
    from concourse.bass2jax import _bass_exec_p

    key = (c, tuple(s.key() for s in segs))
    if key in _EXEC_CACHE:
        return _EXEC_CACHE[key]

    bass2jax.install_neuronx_cc_hook()
    nc = _build_nc(segs)
    if not nc.is_finalized():
        nc.finalize()

    in_names, out_names, out_avals, zero_outs = [], [], [], []
    for alloc in nc.m.functions[0].allocations:
        if not isinstance(alloc, mybir.MemoryLocationSet):
            continue
        name = alloc.memorylocations[0].name
        if alloc.kind == "ExternalInput":
            in_names.append(name)
        elif alloc.kind == "ExternalOutput":
            out_names.append(name)
            shape = tuple(alloc.tensor_shape)
            dtype = mybir.dt.np(alloc.dtype)
            out_avals.append(jax.core.ShapedArray(shape, dtype))
            zero_outs.append(np.zeros(shape, dtype))
    n_params = len(in_names)
    all_names = in_names + out_names
    donate = tuple(range(n_params, n_params + len(out_names)))

    def _body(*args):
        outs = _bass_exec_p.bind(
            *args,
            out_avals=tuple(out_avals),
            in_names=tuple(all_names),
            out_names=tuple(out_names),
            lowering_input_output_aliases=(),
            sim_require_finite=True,
            sim_require_nnan=True,
            nc=nc,
        )
        return tuple(outs)

    dev = jax.devices()[c]
    with jax.default_device(dev):
        jf = jax.jit(_body, donate_argnums=donate, keep_unused=True)
        args = _core_args(nc, in_names, zero_outs, in_map, c)
        comp = jf.lower(*args).compile()
    entry = (comp, nc, in_names, out_names, zero_outs)
    _EXEC_CACHE[key] = entry
    return entry


def _core_args(nc, in_names, zero_outs, in_map, c):
    im = dict(in_map)
    if nc.partition_id_tensor is not None:
        im[nc.partition_id_tensor.name] = np.array([[c]], dtype=np.uint32)
    return [np.asarray(im[n]) for n in in_names] + [z.copy() for z in zero_outs]


def _run(attention_weights, input_lengths, output_lengths, ntff_hook=None):
    attention_weights = np.ascontiguousarray(attention_weights, dtype=np.float32)
    il = np.asarray(input_lengths, dtype=np.int64)
    ol = np.asarray(output_lengths, dtype=np.int64)
    assign, core_segs = _build_schedule(il, ol)
    in_maps = []
    for c in range(N_CORES):
        flat = np.empty(FLAT, np.float32)
        flat[:PADF] = 0.0
        flat[PADF : PADF + B_LOC * T * E] = attention_weights[assign[c]].reshape(-1)
        flat[PADF + B_LOC * T * E :] = 0.0
        in_maps.append(
            {"attn": flat, **_make_tables(il, ol, assign[c], core_segs[c])}
        )

    entries = [
        _get_compiled(c, core_segs[c], in_maps[c]) for c in range(N_CORES)
    ]

    def _dispatch():
        futs = []
        for c, (comp, nc, in_names, out_names, zero_outs) in enumerate(entries):
            args = _core_args(nc, in_names, zero_outs, in_maps[c], c)
            futs.append((comp(*args), out_names))
        return [
            {name: np.asarray(v) for name, v in zip(out_names, outs)}
            for outs, out_names in futs
        ]

    if ntff_hook is not None:
        with ntff_hook:
            results = _dispatch()
    else:
        results = _dispatch()

    total = sum(float(r["acc"].sum(dtype=np.float64)) for r in results)
    total -= _garbage_correction(in_maps, il, ol, assign, core_segs)
    return np.float32(total / float(B * T * E)), results


def kernel(attention_weights, input_lengths, output_lengths):
    out, _ = _run(attention_weights, input_lengths, output_lengths)
    return out
